# revision 51
# baseline (speedup 1.0000x reference)
"""Trainium2 Bass kernel for nn_EstimatorQNNGen104 (dense tiny-MLP over 4.2M rows).

Pure data parallel over 8 NeuronCores (R_core = 524288 rows/core), bf16 data +
fp32r weights.  Per core the batch is processed in 16 "groups" of 2 pairs
(4 supertiles of 8192 rows; banding: partition = 16*feature + rowgroup,
512 cols per band).

Key structure (vs a naive port of the reference):
  - softmax(2) -> sigmoid(d), and every sigmoid is computed as
    0.5 + 0.5*tanh(v/2) with the affine part folded into the next layer's
    weights/biases, so the only activation function used anywhere is tanh.
  - The estimator's first layer consumes a per-ST "combined tile" xc
    [96, 512] = [x (64p) | tanh(c/2) (16p) | tanh(d/2) (16p)] so the whole
    7->8 layer is ONE matmul per supertile (the sampler/conv features are
    copied into xc by cheap DVE tensor_scalar copies).
  - The sampler hidden tanh (4 units) is offloaded off the ScalarE:
    a DVE tensor_scalar drains PSUM with (+bias, max -1) and a single custom
    DVE instruction (deg-7 odd polynomial + upper clamp, 8 ALU stages)
    finishes tanh(3.2*u).  The 1/3.2 scale and the poly's leading
    coefficient fold into the A / B matmul weights.  ScalarE keeps the
    accuracy-critical tanh's (zt, h1-pre, h2-pre) exact; tau-band copies into
    xc run 3x on Pool + 1x on DVE (GPSIMD cannot touch PSUM, SBUF only).
  - E-stage outputs of a group's 2 pairs land in 64 partitions of the pa
    PSUM bank (reused late in the period), drained by one DVE add(eb3).
  - PSUM: pa/pe 1 bank, zt 1, pd 2, pcA 2, pcB 2 = 8 banks exactly.
  - Software pipeline in 16 periods; SCHEDULE gives per-stage period lags
    and per-engine queue order (tuned against TimelineSim); steady state is
    ScalarE-bound at ~4.1us/group.
"""
import numpy as np
from contextlib import ExitStack

import ml_dtypes

B_TOTAL = 4194304
N_CORES = 8
R_CORE = B_TOTAL // N_CORES        # 524288
G = 16
COLS = 512
ST_ROWS = G * COLS                  # 8192
N_ST = R_CORE // ST_ROWS            # 64
N_PAIRS = N_ST // 2                 # 32
N_G = N_PAIRS // 2                  # 16 groups (2 pairs, 4 STs)

CLAMP = 3.2                         # tanh(CLAMP)=0.9967; poly fitted on [-1,1]

BIAS_NAMES = ["ztb", "eb2b", "eb1b", "sab", "eb3b"]

POOL_BUFS = {}
RAMP_SQUEEZE = False
TAIL_SQUEEZE = False
TC_SPLIT = True
WARMUPS = 10
CP_DVE = 2

# (stage, lag): stage for group g=p-lag emitted in period p, in this order.
SCHEDULE = [
    ("dma", -2),
    ("tD", 5), ("tCA", 4), ("tCB", 4), ("zt", 2),
    ("A0", 0), ("A1", 0),
    ("cl0", 0), ("po0", 0), ("cl1", 0), ("po1", 0), ("cp", 2),
    ("D0", 4), ("D1", 4),
    ("C0", 3), ("C1", 3), ("C2", 3), ("C3", 3),
    ("E0", 5), ("E1", 5),
    ("cv0", 1), ("cv1", 1), ("B0", 1), ("B1", 1),
    ("out", 5),
]


# ---------------- poly fit (deg-7 odd, approx-minimax) ----------------

def _fit_tanh7(C):
    u = np.linspace(0, 1, 20001)
    y = np.tanh(C * u)
    A = np.stack([u ** (2 * k + 1) for k in range(4)], axis=1)
    w = np.ones_like(u)
    coef = None
    for _ in range(80):
        coef, *_ = np.linalg.lstsq(A * w[:, None], y * w, rcond=None)
        r = np.abs(A @ coef - y)
        w *= (1.0 + r / (r.max() + 1e-12)) ** 2
        w /= w.max()
    q = np.polynomial.Polynomial(coef)
    roots = q.roots()
    rr = [x for x in roots if abs(x.imag) < 1e-9]
    cc = [x for x in roots if x.imag > 1e-9]
    assert len(rr) == 1 and len(cc) == 1, roots
    r1 = float(rr[0].real)
    p1 = float(-2 * cc[0].real)
    q1 = float(abs(cc[0]) ** 2)
    k = float(coef[-1])
    return k, r1, p1, q1

POLY_K, POLY_R1, POLY_P1, POLY_Q1 = _fit_tanh7(CLAMP)


def _poly_ref(in0, in1, c0, c1, c2):
    # matches the Spec body: v = min(in0, 1); (t-c0)*((t+c1)*t+c2)*v
    v = np.minimum(np.asarray(in0, np.float32), 1.0)
    t = v * v
    return ((t - c0) * ((t + c1) * t + c2) * v).astype(np.float32)


_DVE_OP = [None]


def _get_tanh_op():
    if _DVE_OP[0] is not None:
        return _DVE_OP[0]
    from concourse.dve_spec import (
        Spec, Src0, C0, C1, C2, One, minn, sq, lower, _has_src1,
    )
    from concourse.dve_uop import DveOpSpec
    from concourse.dve_ops import DveOp, OPS, CUSTOM_DVE_SPECS, _SUB_OPCODE_FOR_NAME

    name = "TANH7_ANT_EQNN"
    if name not in _SUB_OPCODE_FOR_NAME:
        v = minn(Src0, One)
        t = sq(v)
        spec = Spec(body=((t - C0) * ((t + C1) * t + C2)) * v,
                    reference=_poly_ref)
        row = max(_SUB_OPCODE_FOR_NAME.values()) + 1
        assert row < 0x20
        _SUB_OPCODE_FOR_NAME[name] = row
        shas = {}
        for ver in ("v3", "v4"):
            s = DveOpSpec(name=name, opcode=row, uops=lower(spec, ver=ver),
                          rd1_en=_has_src1(spec))
            shas[ver] = s.sha(ver)
        op = DveOp(name, spec, subdim=False, uops_sha=shas)
        OPS.append(op)
        CUSTOM_DVE_SPECS[name] = spec
        _DVE_OP[0] = op
    else:
        from concourse.dve_ops import OPS as _ops
        _DVE_OP[0] = next(o for o in _ops if o.name == name)
    return _DVE_OP[0]


# ---------------- host-side weights ----------------

def _build_weights(conv_w, conv_b, sW1, sb1, sW2, sb2,
                   eW1, eb1, eW2, eb2, eW3, eb3):
    """All lhsT matrices [128 or 96, 128] fp32-encoded (fed as float32r)."""
    f64 = np.float64
    conv_w = np.asarray(conv_w, f64).reshape(4)
    sW1, sb1 = np.asarray(sW1, f64), np.asarray(sb1, f64)
    sW2, sb2 = np.asarray(sW2, f64), np.asarray(sb2, f64)
    eW1, eb1 = np.asarray(eW1, f64), np.asarray(eb1, f64)
    eW2, eb2 = np.asarray(eW2, f64), np.asarray(eb2, f64)
    eW3, eb3 = np.asarray(eW3, f64), np.asarray(eb3, f64)

    mats = {}

    # --- A (sampler pre-act / CLAMP): x pair tile -> pa bank.
    # pair tile partitions: 64*half + 16*f + g ; out: 64*half + 16*u + g.
    A = np.zeros((128, 128), f64)
    for h in (0, 1):
        for f in range(2):
            for u in range(4):
                wv = sW1[f, u] / CLAMP
                if wv == 0.0:
                    continue
                for g in range(G):
                    A[64 * h + 16 * f + g, 64 * h + 16 * u + g] = wv
    mats["A0"] = A          # same lhsT for both pairs
    mats["A1"] = A

    # --- CONV (pair kk): x -> zt bands; value c/2 (+bias via ACT bias).
    # zt partitions: 64*kk + 32*h + {0-15: tc, 16-31: td}.
    for kk in (0, 1):
        M = np.zeros((128, 128), f64)
        for h in (0, 1):
            for f in range(4):
                for g in range(G):
                    M[64 * h + 16 * f + g, 64 * kk + 32 * h + 0 + g] = \
                        conv_w[f] * 0.5
        mats[f"CONV{kk}"] = M
    # --- B (pair kk): samp -> zt d-band; d = dw^T tanh + db; samp holds
    # tanh/POLY_K so scale by POLY_K; also *0.5 for the sigma->tanh trick.
    dw = (sW2[:, 0] - sW2[:, 1])
    for kk in (0, 1):
        M = np.zeros((128, 128), f64)
        for h in (0, 1):
            for u in range(4):
                wv = dw[u] * POLY_K * 0.5
                for g in range(G):
                    M[64 * h + 16 * u + g, 64 * kk + 32 * h + 16 + g] = wv
        mats[f"B{kk}"] = M
    # --- C: combined tile [96,512] -> h1 pre (8 out bands, 128 partitions).
    # xc partitions: 0-63: 16f+g (x); 64-79: tc; 80-95: td.
    w_c = eW1[4]
    w_s = eW1[5] - eW1[6]
    C = np.zeros((96, 128), f64)
    for f in range(4):
        for o in range(8):
            wv = eW1[f, o]
            for g in range(G):
                C[16 * f + g, 16 * o + g] = wv
    for o in range(8):
        for g in range(G):
            C[64 + g, 16 * o + g] = 0.5 * w_c[o]
            C[80 + g, 16 * o + g] = 0.5 * w_s[o]
    mats["C"] = C

    # --- D (contract one ST's h1 [8 bands] -> h2 [4 bands]).
    # pd layout per pair: cols [0,512): lo-ST h2 at partitions 0-63,
    # hi-ST h2 at 64-127  -> two lhsT: D_lo (cols 0-63), D_hi (cols 64-127).
    for nm, col0 in (("D_lo", 0), ("D_hi", 64)):
        M = np.zeros((128, 128), f64)
        for f in range(8):
            for o in range(4):
                wv = eW2[f, o]
                for g in range(G):
                    M[16 * f + g, col0 + 16 * o + g] = wv
        mats[nm] = M

    # --- E (pair kk): h2 [128] -> out band; out partitions 32*kk+16*h+g.
    for kk in (0, 1):
        M = np.zeros((128, 128), f64)
        for h in (0, 1):
            for o in range(4):
                wv = eW3[o, 0]
                for g in range(G):
                    M[64 * h + 16 * o + g, 32 * kk + 16 * h + g] = wv
        mats[f"E{kk}"] = M

    # --- biases (fp32, [128,1] each) ---
    biases = {}
    ztb = np.zeros(128, f64)
    for kk in (0, 1):
        for h in (0, 1):
            ztb[64 * kk + 32 * h + 0:64 * kk + 32 * h + 16] = conv_b[0] * 0.5
            ztb[64 * kk + 32 * h + 16:64 * kk + 32 * h + 32] = \
                (sb2[0] - sb2[1]) * 0.5
    biases["ztb"] = ztb
    eb2b = np.zeros(128, f64)
    for h in (0, 1):
        for o in range(4):
            eb2b[64 * h + 16 * o:64 * h + 16 * o + 16] = eb2[o]
    biases["eb2b"] = eb2b
    eb1_eff = eb1 + eW1[6] + 0.5 * w_c + 0.5 * w_s
    eb1b = np.zeros(128, f64)
    for o in range(8):
        eb1b[16 * o:16 * o + 16] = eb1_eff[o]
    biases["eb1b"] = eb1b
    sab = np.zeros(128, f64)
    for h in (0, 1):
        for u in range(4):
            sab[64 * h + 16 * u:64 * h + 16 * u + 16] = sb1[u] / CLAMP
    biases["sab"] = sab
    biases["eb3b"] = np.full(128, eb3[0], f64)

    names = ["A0", "A1", "CONV0", "CONV1", "B0", "B1", "D_lo", "D_hi",
             "E0", "E1"]
    b16 = ml_dtypes.bfloat16
    Wpack = np.stack([mats[n] for n in names], axis=1)       # (128, 10, 128)
    Wpack = np.ascontiguousarray(
        Wpack.reshape(128, len(names) * 128).astype(b16))
    Cpack = np.ascontiguousarray(mats["C"].astype(b16))      # (96,128)
    Bpack = np.ascontiguousarray(np.stack(
        [biases[n] for n in BIAS_NAMES], axis=1).astype(np.float32))
    return Wpack, Cpack, Bpack, names


def _pack_inputs(x):
    """x (R_CORE,4) fp32 -> (pair tiles, xc tiles) in bf16.

    pair tile p: [128, 512], partition 64*h + 16*f + g, col c
                 = x[ST(2p+h) row g*512+c, f]
    xc tile g:   [64, 2048], partition 16*f + gg, col 512*s + c
                 = x[ST(4g+s) row gg*512+c, f]   (s = 2*kk + h)
    """
    xs = x.reshape(N_ST, G, COLS, 4)                  # [st, g, c, f]
    stf = np.ascontiguousarray(xs.transpose(0, 3, 1, 2))  # [st, f, g, c]
    stf = stf.reshape(N_ST, 64, COLS)                 # partition 16f+g
    pair = stf.reshape(N_PAIRS, 2, 64, COLS).reshape(N_PAIRS, 128, COLS)
    xc = stf.reshape(N_G, 4, 64, COLS).transpose(0, 2, 1, 3)
    xc = np.ascontiguousarray(xc.reshape(N_G, 64, 4 * COLS))
    b16 = ml_dtypes.bfloat16
    pairs2 = pair.reshape(N_G, 2, 128, COLS).transpose(0, 2, 1, 3)
    pairs2 = np.ascontiguousarray(pairs2.reshape(N_G, 128, 2 * COLS))
    return (pairs2.astype(b16), xc.astype(b16))


def _unpack_out(y):
    """y (N_G, 64, 512) -> (R_CORE, 1) fp32.
    partition 32*kk + 16*h + g, col c -> ST(4g'+2kk+h) row g*512+c."""
    y = np.asarray(y, np.float32).reshape(N_G, 2, 2, G, COLS)  # [g,kk,h,gg,c]
    y = y.transpose(0, 1, 2, 3, 4)     # already [g, kk, h, gg, c]
    # ST index = 4g + 2kk + h ; rows = st*8192 + gg*512 + c
    return np.ascontiguousarray(y.reshape(R_CORE, 1))


# ---------------- device program ----------------

_CACHED = {}


def _build_program():
    import concourse.bacc as bacc
    import concourse.tile as tile
    from concourse import mybir

    F32 = mybir.dt.float32
    F32R = mybir.dt.float32r
    BF16 = mybir.dt.bfloat16
    AF = mybir.ActivationFunctionType
    ALU = mybir.AluOpType
    tanh_op = _get_tanh_op()

    nc = bacc.Bacc("TRN2", target_bir_lowering=False, debug=False)
    x_d = nc.dram_tensor("X", [N_G, 128, 2 * COLS], BF16,
                         kind="ExternalInput")
    xc_d = nc.dram_tensor("XC", [N_G, 64, 4 * COLS], BF16,
                          kind="ExternalInput")
    w_d = nc.dram_tensor("W", [128, 10 * 128], BF16, kind="ExternalInput")
    wc_d = nc.dram_tensor("WC", [96, 128], BF16, kind="ExternalInput")
    b_d = nc.dram_tensor("BIAS", [128, len(BIAS_NAMES)], F32,
                         kind="ExternalInput")
    y_d = nc.dram_tensor("Y", [N_G, 64, COLS], BF16, kind="ExternalOutput")

    WN = {n: i for i, n in enumerate(
        ["A0", "A1", "CONV0", "CONV1", "B0", "B1", "D_lo", "D_hi",
         "E0", "E1"])}
    BI = {n: i for i, n in enumerate(BIAS_NAMES)}

    with tile.TileContext(nc) as tc, ExitStack() as ctx:
        const = ctx.enter_context(tc.tile_pool(name="const", bufs=1))
        xp = ctx.enter_context(tc.tile_pool(name="xp", bufs=POOL_BUFS.get("xp", 12)))
        xcp = ctx.enter_context(tc.tile_pool(name="xcp", bufs=POOL_BUFS.get("xcp", 8)))
        sap = ctx.enter_context(tc.tile_pool(name="sap", bufs=POOL_BUFS.get("sap", 3)))
        sampp = ctx.enter_context(tc.tile_pool(name="sampp", bufs=POOL_BUFS.get("sampp", 5)))
        ztaup = ctx.enter_context(tc.tile_pool(name="ztaup", bufs=POOL_BUFS.get("ztaup", 3)))
        h1p = ctx.enter_context(tc.tile_pool(name="h1p", bufs=POOL_BUFS.get("h1p", 3)))
        h2p = ctx.enter_context(tc.tile_pool(name="h2p", bufs=POOL_BUFS.get("h2p", 3)))
        accp = ctx.enter_context(tc.tile_pool(name="accp", bufs=POOL_BUFS.get("accp", 3)))
        # PSUM: pa(1, shared with pe) + zt(1) + pd(2) + pc(4) = 8 banks
        pA = ctx.enter_context(tc.tile_pool(name="pA", bufs=1, space="PSUM"))
        pZ = ctx.enter_context(tc.tile_pool(name="pZ", bufs=1, space="PSUM"))
        pD = ctx.enter_context(tc.tile_pool(name="pD", bufs=1, space="PSUM"))
        pCA = ctx.enter_context(tc.tile_pool(name="pCA", bufs=1, space="PSUM"))
        pCB = (ctx.enter_context(tc.tile_pool(name="pCB", bufs=1,
                                              space="PSUM"))
               if TC_SPLIT else pCA)

        bt = const.tile([128, len(BIAS_NAMES)], F32)
        # Warm the ACT table (tanh only) immediately.
        warm = const.tile([128, 1], F32)
        nc.scalar.activation(warm[:], bt[:, 0:1], AF.Tanh)

        wt = const.tile([128, 10 * 128], BF16)
        wct = const.tile([96, 128], BF16)

        # PE p-state pre-warm (cost model: 3us ramp to full speed).
        # memset-backed so the warm matmuls start before any DMA lands.
        pewarm = const.tile([128, 256], BF16)
        nc.vector.memset(pewarm[:], 0.0)
        warm_ps = pCA.tile([128, 1024], F32, tag="pc0")
        for _ in range(WARMUPS):
            nc.tensor.matmul(warm_ps[:, 0:256], pewarm[:, 0:128], pewarm[:],
                             start=True, stop=True, skip_group_check=True)

        xt = {}     # pair -> x tile
        xct = {}    # group -> xc tile

        def dma_x(g, pieces=1):
            # one start for both pairs (HWDGE is ~625ns serial per start)
            t = xp.tile([128, 2 * COLS], BF16, tag="x2")
            nc.sync.dma_start(out=t[:], in_=x_d[g])
            xt[2 * g] = t[:, 0:COLS]
            xt[2 * g + 1] = t[:, COLS:2 * COLS]
            t = xcp.tile([96, 4 * COLS], BF16, tag="xc")
            nc.sync.dma_start(out=t[0:64, :], in_=xc_d[g])
            xct[g] = t

        dma_x(0)
        nc.sync.dma_start(out=wt[:, 0:256], in_=w_d[:, 0:256])
        nc.sync.dma_start(out=bt[:], in_=b_d[:])
        nc.sync.dma_start(out=wt[:, 256:640], in_=w_d[:, 256:640])
        dma_x(1)
        nc.sync.dma_start(out=wct[:], in_=wc_d[:])
        nc.sync.dma_start(out=wt[:, 640:1280], in_=w_d[:, 640:1280])
        dma_x(2)

        def W(name):
            m = WN[name]
            return wt[:, m * 128:(m + 1) * 128]

        def bias(name):
            return bt[:, BI[name]:BI[name] + 1]

        pa_t = {}   # pair -> pa psum tile
        sa_t = {}   # pair -> clamped sbuf tile
        samp = {}   # pair -> poly out (tanh/POLY_K)
        zt_t = {}   # group -> zt psum
        ztau = {}   # group -> tanh(zt) sbuf  [tc|td bands]
        pc_t = {}   # (group, half) -> pc psum [128, 1024]
        pd_t = {}   # group -> pd psum [128, 1024] (pair0|pair1)
        h1 = {}     # (group, half) -> h1 sbuf [128, 1024]
        h2 = {}     # group -> h2 sbuf [128, 1024]
        pe_t = {}   # group -> pe psum (pa pool, 64 partitions)

        def emit_A(g, kk):
            p = 2 * g + kk
            pa = pA.tile([128, COLS], F32, tag="pa")
            pa_t[p] = pa
            nc.tensor.matmul(pa[:], W(f"A{kk}"), xt[p][:],
                             start=True, stop=True, skip_group_check=True)

        def emit_clamp(g, kk):
            # DVE: sa = max(pa + sab, -1.0)  (upper clamp inside the DVE op;
            # GPSIMD cannot read PSUM, so this drain must be DVE)
            p = 2 * g + kk
            t = sap.tile([128, COLS], BF16, tag="sa")
            sa_t[p] = t
            nc.vector.tensor_scalar(t[:], pa_t[p][:], bias("sab"), -1.0,
                                    ALU.add, ALU.max)
            del pa_t[p]

        def emit_poly(g, kk):
            p = 2 * g + kk
            t = sampp.tile([128, COLS], BF16, tag="samp")
            samp[p] = t
            nc.vector._custom_dve(tanh_op, out=t[:], in0=sa_t[p][:],
                                  s0=POLY_R1, s1=POLY_P1, imm2=POLY_Q1)
            del sa_t[p]

        def emit_conv(g, kk):
            p = 2 * g + kk
            if kk == 0:
                zt = pZ.tile([128, COLS], F32, tag="pz")
                zt_t[g] = zt
            nc.tensor.matmul(zt_t[g][:], W(f"CONV{kk}"), xt[p][:],
                             start=(kk == 0), stop=False,
                             skip_group_check=True)
            del xt[p]
            # underlying [128,1024] tile is freed once both halves are deleted

        def emit_B(g, kk):
            nc.tensor.matmul(zt_t[g][:], W(f"B{kk}"), samp[2 * g + kk][:],
                             start=False, stop=(kk == 1),
                             skip_group_check=True)
            del samp[2 * g + kk]

        def emit_tanh_zt(g):
            t = ztaup.tile([128, COLS], BF16, tag="ztau")
            ztau[g] = t
            nc.scalar.activation(t[:], zt_t[g][:], AF.Tanh, bias=bias("ztb"))
            del zt_t[g]

        def emit_copies(g):
            # 4 copies [32,512]: ztau bands -> xc partitions 64-95.
            # 3 on Pool (SBUF->SBUF is legal there), 1 on DVE for balance.
            for s in range(4):
                eng = nc.vector if s >= 4 - CP_DVE else nc.gpsimd
                eng.tensor_scalar(
                    xct[g][64:96, s * COLS:(s + 1) * COLS],
                    ztau[g][32 * s:32 * s + 32, :], 1.0, None, ALU.mult)
            del ztau[g]

        def emit_C(g, s, alloc=False):
            if not TC_SPLIT:
                if s == 0:
                    pct = pCA.tile([128, 4 * COLS], F32, tag="pc0")
                    pc_t[(g, 0)] = pc_t[(g, 1)] = pct
                nc.tensor.matmul(pc_t[(g, 0)][:, s * COLS:(s + 1) * COLS],
                                 wct[:], xct[g][:, s * COLS:(s + 1) * COLS],
                                 start=True, stop=True,
                                 skip_group_check=True)
                if s == 3:
                    del xct[g]
                return
            hh = s // 2
            if s % 2 == 0:
                pool = pCA if hh == 0 else pCB
                pct = pool.tile([128, 2 * COLS], F32, tag=f"pc{hh}")
                pc_t[(g, hh)] = pct
            nc.tensor.matmul(pc_t[(g, hh)][:, (s % 2) * COLS:
                                           (s % 2 + 1) * COLS],
                             wct[:], xct[g][:, s * COLS:(s + 1) * COLS],
                             start=True, stop=True, skip_group_check=True)
            if s == 3:
                del xct[g]

        def emit_tanhC(g, hh):
            if not TC_SPLIT:
                if hh == 1:
                    return
                t = h1p.tile([128, 4 * COLS], BF16, tag="h1")
                h1[(g, 0)] = t[:, 0:2 * COLS]
                h1[(g, 1)] = t[:, 2 * COLS:4 * COLS]
                nc.scalar.activation(t[:], pc_t[(g, 0)][:], AF.Tanh,
                                     bias=bias("eb1b"))
                del pc_t[(g, 0)], pc_t[(g, 1)]
                return
            t = h1p.tile([128, 2 * COLS], BF16, tag="h1")
            h1[(g, hh)] = t
            nc.scalar.activation(t[:], pc_t[(g, hh)][:], AF.Tanh,
                                 bias=bias("eb1b"))
            del pc_t[(g, hh)]

        def emit_D(g, kk):
            if kk == 0:
                pd = pD.tile([128, 2 * COLS], F32, tag="pd")
                pd_t[g] = pd
            nc.tensor.matmul(pd_t[g][:, kk * COLS:(kk + 1) * COLS],
                             W("D_lo"), h1[(g, kk)][:, 0:COLS],
                             start=True, stop=False, skip_group_check=True)
            nc.tensor.matmul(pd_t[g][:, kk * COLS:(kk + 1) * COLS],
                             W("D_hi"), h1[(g, kk)][:, COLS:2 * COLS],
                             start=False, stop=True, skip_group_check=True)
            del h1[(g, kk)]

        def emit_tanhD(g):
            t = h2p.tile([128, 2 * COLS], BF16, tag="h2")
            h2[g] = t
            nc.scalar.activation(t[:], pd_t[g][:], AF.Tanh, bias=bias("eb2b"))
            del pd_t[g]

        def emit_E(g, kk):
            if kk == 0:
                pe = pA.tile([128, COLS], F32, tag="pa")
                pe_t[g] = pe
            nc.tensor.matmul(pe_t[g][0:64, :], W(f"E{kk}")[:, 0:64],
                             h2[g][:, kk * COLS:(kk + 1) * COLS],
                             start=(kk == 0), stop=(kk == 1),
                             skip_group_check=True)
            if kk == 1:
                del h2[g]

        def emit_out(g):
            acc = accp.tile([64, COLS], BF16, tag="acc")
            nc.vector.tensor_scalar(acc[:], pe_t[g][0:64, :],
                                    bt[0:64, BI["eb3b"]:BI["eb3b"] + 1], None,
                                    ALU.add)
            nc.sync.dma_start(out=y_d[g], in_=acc[:])
            del pe_t[g]

        # ---- software pipeline (emission order = per-engine queue order) ----
        stages = {
            "A0": lambda g: emit_A(g, 0),
            "A1": lambda g: emit_A(g, 1),
            "cl0": lambda g: emit_clamp(g, 0),
            "cl1": lambda g: emit_clamp(g, 1),
            "po0": lambda g: emit_poly(g, 0),
            "po1": lambda g: emit_poly(g, 1),
            "cv0": lambda g: emit_conv(g, 0),
            "cv1": lambda g: emit_conv(g, 1),
            "B0": lambda g: emit_B(g, 0),
            "B1": lambda g: emit_B(g, 1),
            "zt": emit_tanh_zt,
            "cp": emit_copies,
            "C0": lambda g: emit_C(g, 0, alloc=True),
            "C1": lambda g: emit_C(g, 1),
            "C2": lambda g: emit_C(g, 2),
            "C3": lambda g: emit_C(g, 3),
            "tCA": lambda g: emit_tanhC(g, 0),
            "tCB": lambda g: emit_tanhC(g, 1),
            "D0": lambda g: emit_D(g, 0),
            "D1": lambda g: emit_D(g, 1),
            "tD": emit_tanhD,
            "E0": lambda g: emit_E(g, 0),
            "E1": lambda g: emit_E(g, 1),
            "out": emit_out,
        }
        # shallow (dependency-minimal) lag per stage for ramp/tail squeeze
        SHALLOW = {"A0": 0, "A1": 0, "cl0": 0, "cl1": 0, "po0": 0, "po1": 0,
                   "cv0": 0, "cv1": 0, "B0": 0, "B1": 0, "zt": 1, "cp": 1,
                   "C0": 1, "C1": 1, "C2": 1, "C3": 1, "tCA": 2, "tCB": 2,
                   "D0": 2, "D1": 2, "tD": 3, "E0": 3, "E1": 3, "out": 3}

        def lag_eff(name, lag, g):
            if name == "dma":
                return lag
            s = SHALLOW[name]
            if RAMP_SQUEEZE and TAIL_SQUEEZE:
                return min(lag, s + g, s + (N_G - 1 - g))
            if RAMP_SQUEEZE:
                return min(lag, s + g)
            if TAIL_SQUEEZE:
                return min(lag, s + (N_G - 1 - g))
            return lag

        max_lag = max(lag for _, lag in SCHEDULE)
        for p in range(N_G + max_lag + 1):
            for name, lag in SCHEDULE:
                if name == "dma":
                    g = p - lag
                    if 3 <= g < N_G:
                        dma_x(g)
                    continue
                for g in range(N_G):
                    if g + lag_eff(name, lag, g) == p:
                        stages[name](g)

    nc.compile()
    return nc


def kernel(**inputs):
    from concourse.bass_utils import run_bass_kernel_spmd

    inputs = {k: np.asarray(v, np.float32) for k, v in inputs.items()}
    x = inputs["inputs"]
    Wpack, Cpack, Bpack, _ = _build_weights(
        inputs["conv_w"], inputs["conv_b"], inputs["sW1"], inputs["sb1"],
        inputs["sW2"], inputs["sb2"], inputs["eW1"], inputs["eb1"],
        inputs["eW2"], inputs["eb2"], inputs["eW3"], inputs["eb3"])

    if "nc" not in _CACHED:
        _CACHED["nc"] = _build_program()
    nc = _CACHED["nc"]

    in_maps = []
    for c in range(N_CORES):
        xc = x[c * R_CORE:(c + 1) * R_CORE]
        pair, xcomb = _pack_inputs(xc)
        in_maps.append({"X": pair, "XC": xcomb, "W": Wpack, "WC": Cpack,
                        "BIAS": Bpack})

    res = run_bass_kernel_spmd(nc, in_maps, list(range(N_CORES)))
    out = np.concatenate(
        [_unpack_out(res.results[c]["Y"]) for c in range(N_CORES)], axis=0)
    return out.astype(np.float32)
